# revision 1
# baseline (speedup 1.0000x reference)
"""Trainium2 Bass kernel for nn_ConvLayerWithStyleMod.

Math: the reference is (per-sample style-modulated 3x3 conv_transpose stride 2)
followed by a fixed 4x4 blur. Both are convolutions, so they compose into a
single 6x6 kernel applied to the 2x-dilated input. Splitting the 256x256 output
by (row, col) parity turns that into FOUR dense 3x3 SAME convolutions on the
original 128x128 grid (in-ch 128 -> out-ch 64), which is pure TensorEngine work:
9 shifted matmuls per phase, phases paired two-at-a-time into M=128 matmuls.

Sharding: data-parallel over batch; sample i runs on core i (B == 8 == n_cores).
The tiny per-sample weight modulation/demodulation + blur composition is done
on host (float64) and shipped as a per-core weight tensor. x is shipped
zero-padded (130x130) so no on-device memsets are needed; everything the
matmul touches is float32r end to end (full-rate PE path).
"""

import math

import numpy as np

B, C, OC, SD, H = 8, 128, 64, 512, 128
KW = 3
EPS = 1e-8
HP = H + 2          # zero-padded image size in SBUF
NCORES = 8
RB = 4              # image rows per matmul chunk (N = RB*H = 512)
G = 8               # chunks in flight per group (= psum banks)

_prog = None


def _host_phase_weights(style, weight, mod_weight, mod_bias):
    """Per-sample phase kernels, laid out as matmul lhsT.

    Returns (B, C, 2*9*128) float32 where
    wts[b][c, (pc*9 + tap)*128 + pr*64 + o] = Kp[b, pr, pc, o, c, dr+1, dc+1],
    tap = (dr+1)*3 + (dc+1).
    """
    style = np.asarray(style, dtype=np.float64)
    weight = np.asarray(weight, dtype=np.float64)
    mod_weight = np.asarray(mod_weight, dtype=np.float64)
    mod_bias = np.asarray(mod_bias, dtype=np.float64)

    b = style.shape[0]
    scale = 1.0 / math.sqrt(C * KW * KW)
    w_gain = 1.0 / math.sqrt(mod_weight.shape[1])
    s = style @ (mod_weight.T * w_gain) + mod_bias            # (b, C)
    wmod = scale * weight * s[:, None, :, None, None]          # (b, OC, C, 3, 3)
    demod = 1.0 / np.sqrt(np.sum(wmod * wmod, axis=(2, 3, 4)) + EPS)
    wmod = wmod * demod[:, :, None, None, None]
    wt = wmod[:, :, :, ::-1, ::-1]

    fir = np.array([1.0, 3.0, 3.0, 1.0])
    kern = np.outer(fir, fir)
    kern = kern / kern.sum() * 4.0
    blurk = kern[::-1, ::-1]

    keff = np.zeros((b, OC, C, 6, 6))
    for u in range(3):
        for v in range(3):
            keff[:, :, :, u:u + 4, v:v + 4] += wt[:, :, :, u:u + 1, v:v + 1] * blurk

    rowsel = {0: slice(1, None, 2), 1: slice(0, None, 2)}
    wts = np.zeros((b, C, 2 * 9 * 128), dtype=np.float32)
    for pc in range(2):
        for pr in range(2):
            # (b, OC, C, 3, 3) for this phase
            kp = keff[:, :, :, rowsel[pr], :][:, :, :, :, rowsel[pc]]
            for dr in range(3):
                for dc in range(3):
                    tap = dr * 3 + dc
                    col0 = (pc * 9 + tap) * 128 + pr * 64
                    # [b, C, OC]
                    wts[:, :, col0:col0 + OC] = kp[:, :, :, dr, dc].transpose(0, 2, 1)
    return wts


def _build():
    import concourse.bacc as bacc
    import concourse.mybir as mybir
    from concourse.tile import TileContext

    f32 = mybir.dt.float32
    f32r = mybir.dt.float32r

    nc = bacc.Bacc(None, target_bir_lowering=False)
    x = nc.declare_dram_parameter("x", [C, HP * HP], f32r, isOutput=False)
    wts = nc.declare_dram_parameter("wts", [C, 2 * 9 * 128], f32r, isOutput=False)
    out = nc.declare_dram_parameter("out", [OC, 2 * H, 2 * H], f32, isOutput=True)

    with TileContext(nc) as tc:
        with (
            tc.tile_pool(name="xp", bufs=1) as xpool,
            tc.tile_pool(name="wp", bufs=1) as wpool,
            tc.tile_pool(name="ps", bufs=8, space="PSUM") as pspool,
            tc.tile_pool(name="ob", bufs=4) as opool,
        ):
            xpad = xpool.tile([C, HP * HP], f32r)
            xv = xpad[:, :].rearrange("p (r c) -> p r c", c=HP)
            wtile = wpool.tile([C, 2 * 9 * 128], f32r)
            nc.sync.dma_start(out=wtile[:, :], in_=wts[:, :])

            # x arrives pre-padded; load in row-band slices so early chunks
            # can start compute before the whole image lands
            xdram = x.rearrange("p (r c) -> p r c", c=HP)
            NSLC = 32
            rs = HP // NSLC  # 16
            for sl in range(NSLC):
                r0 = sl * rs
                r1 = HP if sl == NSLC - 1 else r0 + rs
                nc.sync.dma_start(
                    out=xv[:, r0:r1, :],
                    in_=xdram[:, r0:r1, :],
                )

            nchunks = H // RB
            for g0 in range(0, nchunks, G):
                osb = [
                    opool.tile([C, RB, 2 * H], f32, tag="osb", name=f"osb{g0}_{i}")
                    for i in range(G)
                ]
                for pc in range(2):
                    ps = [
                        pspool.tile([C, RB, H], f32, tag="ps", name=f"ps{g0}_{pc}_{i}")
                        for i in range(G)
                    ]
                    for tap in range(9):
                        dr, dc = tap // 3 - 1, tap % 3 - 1
                        w_ap = wtile[:, (pc * 9 + tap) * 128:(pc * 9 + tap + 1) * 128]
                        for cg in range(G):
                            a0 = (g0 + cg) * RB
                            rhs = xv[:, a0 + dr + 1:a0 + dr + 1 + RB, 1 + dc:1 + dc + H]
                            nc.tensor.matmul(
                                ps[cg][:, :, :],
                                w_ap,
                                rhs,
                                start=(tap == 0),
                                stop=(tap == 8),
                            )
                    for cg in range(G):
                        nc.vector.tensor_copy(
                            out=osb[cg][:, :, pc::2], in_=ps[cg][:, :, :]
                        )
                for cg in range(G):
                    a0 = (g0 + cg) * RB
                    nc.sync.dma_start(
                        out=out[:, 2 * a0:2 * a0 + 2 * RB:2, :],
                        in_=osb[cg][0:OC, :, :],
                    )
                    nc.sync.dma_start(
                        out=out[:, 2 * a0 + 1:2 * a0 + 2 * RB:2, :],
                        in_=osb[cg][OC:2 * OC, :, :],
                    )
    nc.compile()
    return nc


def _get_prog():
    global _prog
    if _prog is None:
        _prog = _build()
    return _prog


def _pad_x(xi):
    xp = np.zeros((C, HP, HP), dtype=np.float32)
    xp[:, 1:1 + H, 1:1 + H] = xi
    return xp.reshape(C, HP * HP)


def kernel(x, style, weight, mod_weight, mod_bias):
    from concourse.bass_utils import run_bass_kernel_spmd

    nc = _get_prog()
    wts = _host_phase_weights(style, weight, mod_weight, mod_bias)
    x = np.asarray(x, dtype=np.float32)
    in_maps = [
        {"x": _pad_x(x[i]), "wts": np.ascontiguousarray(wts[i])}
        for i in range(NCORES)
    ]
    r = run_bass_kernel_spmd(nc, in_maps, list(range(NCORES)))
    return np.stack([r.results[i]["out"] for i in range(NCORES)], axis=0)



# revision 4
# speedup vs baseline: 1.7929x; 1.7929x over previous
"""Trainium2 Bass kernel for nn_ConvLayerWithStyleMod.

Math: reference = per-sample style-modulated 3x3 conv_transpose (stride 2)
followed by a fixed separable 4x4 blur ([1,3,3,1] outer [1,3,3,1]).

Decomposition used here (validated vs the reference to ~4e-7):
 - Fold ONLY the vertical blur axis into the conv weights. Splitting the
   dilated output grid by (row parity pr, col parity rc) leaves, per output
   row-parity, two column-parity planes Z0 (even dilated cols, 6 taps) and
   Z1 (odd dilated cols, 3 taps) -- 9 shifted matmuls per row chunk instead
   of the 18 a full 2D fold needs. Both pr phases pack into M=128.
 - The horizontal blur [1,3,3,1] = [1,1]*[1,1]*[1,1] (binomial) is three
   2-tap add stages on the cheap engines, in parity form:
     A0 = Z0[b] + Z1[b]         A1 = Z1[b] + Z0[b+1]
     B0 = A0[b] + A1[b]         B1 = A1[b] + A0[b+1]
     out0[b] = B1[b-1] + B0[b]  out1[b] = B0[b] + B1[b]
   (a global x0.25 for the two blur-axis normalizations is baked into the
   host-folded weights).
 - fp16 for x / weights / blur intermediates (PE rate is identical to f32r,
   DMA-in halves, DVE adds get the 2x packed mode); PSUM accum + final
   output stay f32.

Sharding: data-parallel over batch; sample i on core i. Output is written
as [p=(pr,o), a, c] (128 partitions) so each chunk is ONE full-width DMA;
host de-interleaves rows at the end.
"""

import math

import numpy as np

B, C, OC, SD, H = 8, 128, 64, 512, 128
KW = 3
EPS = 1e-8
HP = H + 2          # zero-padded image size in SBUF (rows/cols -1..128)
NCORES = 8
RB = 3              # image rows per chunk (PSUM bank: 3*130 = 390 <= 512 f32)
NTAP = 9            # 6 Z0 taps + 3 Z1 taps

_prog = None


def _host_z_kernels(style, weight, mod_weight, mod_bias):
    """Per-sample vertically-blur-folded kernels as matmul lhsT, f16.

    Returns (B, C, 9*128) float16 where tap t column block t*128 + pr*64 + o:
      t = d*2 + e (d row tap 0..2, e col shift 0..1)  -> Z0 taps
      t = 6 + d                                        -> Z1 taps
    """
    style = np.asarray(style, dtype=np.float64)
    weight = np.asarray(weight, dtype=np.float64)
    mod_weight = np.asarray(mod_weight, dtype=np.float64)
    mod_bias = np.asarray(mod_bias, dtype=np.float64)

    b = style.shape[0]
    scale = 1.0 / math.sqrt(C * KW * KW)
    w_gain = 1.0 / math.sqrt(mod_weight.shape[1])
    s = style @ (mod_weight.T * w_gain) + mod_bias             # (b, C)
    wmod = scale * weight * s[:, None, :, None, None]          # (b, OC, C, 3, 3)
    demod = 1.0 / np.sqrt(np.sum(wmod * wmod, axis=(2, 3, 4)) + EPS)
    wt = wmod * demod[:, :, None, None, None]                  # conv_transpose taps

    # vertical blur fold: out dilated row J = 2a+pr reads x row a+dlt with
    # FIR tap u = 2*dlt + dr + 1 - pr ; f1 = [1,3,3,1]/4, extra 0.25 for the
    # unscaled horizontal binomial stages.
    f1 = np.array([1.0, 3.0, 3.0, 1.0]) / 4.0
    V = np.zeros((2, 3, 3))
    for pr in range(2):
        for dlt in (-1, 0, 1):
            for dr in range(3):
                u = 2 * dlt + dr + 1 - pr
                if 0 <= u <= 3:
                    V[pr, dlt + 1, dr] += f1[u] * 0.25

    wts = np.zeros((b, C, NTAP * 128), dtype=np.float64)
    # wt[b, o, c, dr, dc] -> per tap block [c, pr*64+o]
    wtT = wt.transpose(0, 2, 1, 3, 4)                          # (b, C, OC, 3, 3)
    for pr in range(2):
        for d in range(3):
            # Z0: dc = 2e (e = col shift); Z1: dc = 1
            acc0 = np.zeros((b, C, OC, 2))
            acc1 = np.zeros((b, C, OC))
            for dr in range(3):
                v = V[pr, d, dr]
                if v == 0.0:
                    continue
                acc0[..., 0] += v * wtT[:, :, :, dr, 0]
                acc0[..., 1] += v * wtT[:, :, :, dr, 2]
                acc1 += v * wtT[:, :, :, dr, 1]
            for e in range(2):
                t = d * 2 + e
                wts[:, :, t * 128 + pr * 64:t * 128 + pr * 64 + OC] = acc0[..., e]
            t = 6 + d
            wts[:, :, t * 128 + pr * 64:t * 128 + pr * 64 + OC] = acc1
    return wts.astype(np.float16)


def _build():
    import concourse.bacc as bacc
    import concourse.mybir as mybir
    from concourse.tile import TileContext

    f32 = mybir.dt.float32
    f16 = mybir.dt.float16
    AOp = mybir.AluOpType

    nc = bacc.Bacc(None, target_bir_lowering=False)
    x = nc.declare_dram_parameter("x", [C, HP * HP], f16, isOutput=False)
    wts = nc.declare_dram_parameter("wts", [C, NTAP * 128], f16, isOutput=False)
    # out[p = pr*64 + o, a, c]; host interleaves rows (2a+pr) afterwards
    out = nc.declare_dram_parameter("out", [128, H, 2 * H], f32, isOutput=True)

    with TileContext(nc) as tc:
        with (
            tc.tile_pool(name="xp", bufs=1) as xpool,
            tc.tile_pool(name="wp", bufs=1) as wpool,
            tc.tile_pool(name="ps", bufs=4, space="PSUM") as pspool,
            tc.tile_pool(name="zc", bufs=3) as zcpool,
            tc.tile_pool(name="ab", bufs=3) as abpool,
            tc.tile_pool(name="ob", bufs=4) as opool,
        ):
            xpad = xpool.tile([C, HP * HP], f16)
            xv = xpad[:, :].rearrange("p (r c) -> p r c", c=HP)
            wtile = wpool.tile([C, NTAP * 128], f16)
            nc.sync.dma_start(out=wtile[:, :], in_=wts[:, :])

            # x arrives pre-padded; band loads so compute starts early
            xdram = x.rearrange("p (r c) -> p r c", c=HP)
            NB = 13
            rs = 10
            for sl in range(NB):
                r0 = sl * rs
                r1 = HP if sl == NB - 1 else r0 + rs
                nc.sync.dma_start(out=xv[:, r0:r1, :], in_=xdram[:, r0:r1, :])

            nchunks = (H + RB - 1) // RB
            for ci in range(nchunks):
                a0 = ci * RB
                rb = min(RB, H - a0)
                z0 = pspool.tile([C, RB, H + 1], f32, tag="z0", name=f"z0_{ci}")
                z1 = pspool.tile([C, RB, H + 2], f32, tag="z1", name=f"z1_{ci}")
                # Z0: 6 taps (d rows x e col-shifts); x col idx b-e+1
                for d in range(3):
                    for e in range(2):
                        t = d * 2 + e
                        nc.tensor.matmul(
                            z0[:, 0:rb, :],
                            wtile[:, t * 128:(t + 1) * 128],
                            xv[:, a0 + d:a0 + d + rb, 1 - e:HP - e - 1 + 1],
                            start=(t == 0),
                            stop=(t == 5),
                        )
                # Z1: 3 taps; full padded col range
                for d in range(3):
                    t = 6 + d
                    nc.tensor.matmul(
                        z1[:, 0:rb, :],
                        wtile[:, t * 128:(t + 1) * 128],
                        xv[:, a0 + d:a0 + d + rb, 0:HP],
                        start=(d == 0),
                        stop=(d == 2),
                    )
                # PSUM -> SBUF f16 casts on the (otherwise idle) ACT engine
                z0c = zcpool.tile([C, RB, H + 1], f16, tag="z0c", name=f"z0c_{ci}")
                z1c = zcpool.tile([C, RB, H + 2], f16, tag="z1c", name=f"z1c_{ci}")
                nc.scalar.copy(z0c[:, 0:rb, :], z0[:, 0:rb, :])
                nc.scalar.copy(z1c[:, 0:rb, :], z1[:, 0:rb, :])
                # binomial blur stages (DVE, f16 packed 2x)
                A0 = abpool.tile([C, RB, H + 1], f16, tag="A0", name=f"A0_{ci}")
                A1 = abpool.tile([C, RB, H + 1], f16, tag="A1", name=f"A1_{ci}")
                B0 = abpool.tile([C, RB, H], f16, tag="B0", name=f"B0_{ci}")
                B1 = abpool.tile([C, RB, H + 1], f16, tag="B1", name=f"B1_{ci}")
                nc.vector.tensor_tensor(
                    out=A0[:, 0:rb, :], in0=z0c[:, 0:rb, :],
                    in1=z1c[:, 0:rb, 1:H + 2], op=AOp.add)
                nc.vector.tensor_tensor(
                    out=A1[:, 0:rb, :], in0=z1c[:, 0:rb, 0:H + 1],
                    in1=z0c[:, 0:rb, :], op=AOp.add)
                nc.vector.tensor_tensor(
                    out=B0[:, 0:rb, :], in0=A0[:, 0:rb, 0:H],
                    in1=A1[:, 0:rb, 1:H + 1], op=AOp.add)
                nc.vector.tensor_tensor(
                    out=B1[:, 0:rb, :], in0=A1[:, 0:rb, :],
                    in1=A0[:, 0:rb, :], op=AOp.add)
                # final interleaved f32 writes; Pool can't run TensorScalarPtr
                # so these are plain adds, load-balanced DVE/Pool
                osb = opool.tile([C, RB, 2 * H], f32, tag="osb", name=f"osb_{ci}")
                eng0 = nc.vector if ci % 2 == 0 else nc.gpsimd
                eng0.tensor_tensor(
                    out=osb[:, 0:rb, 0::2], in0=B1[:, 0:rb, 0:H],
                    in1=B0[:, 0:rb, :], op=AOp.add)
                nc.gpsimd.tensor_tensor(
                    out=osb[:, 0:rb, 1::2], in0=B0[:, 0:rb, :],
                    in1=B1[:, 0:rb, 1:H + 1], op=AOp.add)
                nc.sync.dma_start(
                    out=out[:, a0:a0 + rb, :], in_=osb[:, 0:rb, :])
    nc.compile()
    return nc


def _get_prog():
    global _prog
    if _prog is None:
        _prog = _build()
    return _prog


def _pad_x(xi):
    xp = np.zeros((C, HP, HP), dtype=np.float16)
    xp[:, 1:1 + H, 1:1 + H] = xi
    return xp.reshape(C, HP * HP)


def kernel(x, style, weight, mod_weight, mod_bias):
    from concourse.bass_utils import run_bass_kernel_spmd

    nc = _get_prog()
    wts = _host_z_kernels(style, weight, mod_weight, mod_bias)
    x = np.asarray(x)
    in_maps = [
        {"x": _pad_x(x[i]), "wts": np.ascontiguousarray(wts[i])}
        for i in range(NCORES)
    ]
    r = run_bass_kernel_spmd(nc, in_maps, list(range(NCORES)))
    outs = []
    for i in range(NCORES):
        o = r.results[i]["out"]                    # (128, 128, 256) [pr*64+o, a, c]
        o = o.reshape(2, OC, H, 2 * H).transpose(1, 2, 0, 3)
        outs.append(o.reshape(OC, 2 * H, 2 * H))
    return np.stack(outs, axis=0).astype(np.float32)
